# revision 19
# baseline (speedup 1.0000x reference)
"""ContainmentLoss Trainium2 kernel.

Mathematical collapse exploited: the reference's 256-iteration cascaded-conv
distance transform converges after its FIRST iteration for any input whose
`outside` map is strictly positive (true for sigmoid outputs): the 3x3 kernel
has center weight 1.0, so any pixel that fires (conv < 1) has its boundary
snapped to 1, forcing conv >= 1 forever after; conv is monotone non-decreasing
so pixels with conv >= 1 at iter 0 never fire.  Hence

    dist    = relu(-0.35 * ln(conv3x3(outside)))        (offset_0 = 0)
    penalty = min(dist, 10) / 10     (the min never binds: conv >= 0.03)
    loss    = mean(pred[:,1] * outside * penalty)

with outside = 1 - dilate5x5(sigmoid(10*(target[:,0]-0.5)))
             = 1 / (1 + exp(10*maxpool5x5(target[:,0]) - 5))   (monotonicity)

Sharding: 8 cores; core c handles image b=c//2, row-half h=c%2 (128 rows).
Device layout is transposed (partitions = image columns, free dim packs the
two 128-column halves x rows) so all row-direction windows/halos live in the
free dimension.  The column-direction 5-tap max comes from strided window
loads of the host-prepped transposed slab (the 5 partition-shifted copies of
one column window are a contiguous 670-float run in the slab, so the loads
are cheap); the tree is split per column-half across DVE (h0) and Pool/gpsimd
(h1) so the two halves run in parallel.  The conv's column combine is TWO
float32r matmuls on the (otherwise idle) PE array accumulating in PSUM:

    conv = S1 @ s2 + S2 @ oc,   S1 = kb*Sh + ka*I,  S2 = ka*Sh + I,

where Sh is the +-1 off-diagonal shift matrix (host-provided, symmetric) and
s2[r] = o[r-1] + o[r+1] is the row-direction pair sum.  This replaces the
baseline's SBUF->SBUF partition-shift DMAs (~2us of dead latency) and the
whole P/Q row-conv block on DVE.

The 4 column-edge cases per core (w = 0, 127, 128, 255 where the partition
shift wraps across half tiles or the image border) are NOT fixed on device;
instead the device exports its per-column partial sums plus the 4 boundary
rows of `outside`, and the host recomputes those 4 columns exactly.

Hardware constraint honored throughout: each instruction may carry at most
ONE attached sync wait, so every op has at most one not-yet-observed
dependency; tiny "touch" copies advance engine clocks where needed (one on
ACT before the full-width exp, one warm-up matmul on PE to observe the
shift-matrix DMA), and the Tile kernel-tail drain is split into one
single-wait drain per semaphore.
"""

from contextlib import ExitStack

import numpy as np

import bass_rust
import concourse.bass as bass
import concourse.mybir as mybir
from concourse import tile
from concourse.bass_utils import run_bass_kernel_spmd

F32 = mybir.dt.float32
F32R = mybir.dt.float32r
BF16 = mybir.dt.bfloat16
AF = mybir.ActivationFunctionType
ALU = mybir.AluOpType

B, C, H, W = 4, 5, 256, 256
N_CORES = 8
DT_H = 0.35
KA = float(np.exp(-1.0 / DT_H))           # edge-adjacent kernel weight
KB = float(np.exp(-np.sqrt(2.0) / DT_H))  # diagonal kernel weight
NEG = -1.0e30                             # stand-in for -inf (finite-safe)

_NC_CACHE = None


class _OneWaitTileContext(tile.TileContext):
    """TileContext whose kernel-tail quiesce respects the 1-wait-per-
    instruction limit of this walrus: emit one single-wait drain per
    outstanding semaphore instead of one drain carrying them all."""

    def _drain_and_barrier(self, tick_clock, wait_clock):
        from concourse.vector_clock import ScopedClock

        drain_inst = self.nc.sync.drain()
        wait_clock.add_sem_waits(
            drain_inst.ins, ScopedClock({None: tick_clock.global_clock})
        )
        si = drain_inst.ins.sync_info
        if si is not None and len(si.on_wait) > 1:
            waits = list(si.on_wait)
            drain_inst.ins.sync_info = bass_rust.SyncInfo(
                on_wait=[waits[0]], on_update=list(si.on_update)
            )
            # spread the remaining single-wait drains across engines so they
            # run in parallel (8 serial SP drains cost ~800ns otherwise)
            engines = [self.nc.vector, self.nc.scalar, self.nc.gpsimd,
                       self.nc.tensor, self.nc.sync]
            for i, w in enumerate(waits[1:]):
                d2 = engines[i % len(engines)].drain()
                d2.ins.sync_info = bass_rust.SyncInfo(on_wait=[w], on_update=[])

        self.nc.all_engine_barrier()
        assert self.sems is not None
        popped = self.nc._tile_sem_poison_stack.pop()
        assert popped is self._sem_poison
        self._clear_sems_one_by_one(list(self.sems.allocated().values()))

    def _clear_sems_one_by_one(self, sems):
        """clear_and_free_semaphores, but with per-sem EventSemaphore
        sem-wr-imm writes: this walrus rejects the RANGE_CLEAR InstISA
        ("ISA wrong length")."""
        from concourse.bass import SemaphoreHandle, compact_to_ranges
        if not sems:
            return
        nc = self.nc
        sem_nums = [s.num if isinstance(s, SemaphoreHandle) else s for s in sems]
        for sem_range in compact_to_ranges(sem_nums):
            assert nc._state.free_isdisjoint(sem_range)
            nc.gpsimd.dma_reset(sem_range)
        for s in sems:
            inst = nc.gpsimd.sem_inc(s, 0)
            u = inst.ins.sync_info.on_update[0]
            inst.ins.sync_info = bass_rust.SyncInfo(on_wait=[], on_update=[
                bass_rust.SyncUpdate(
                    sync_type='semaphore', id=u.id, ant_name=u.ant_name,
                    update_mode='sem-wr-imm', update_value=0,
                    update_reg=None)])
        nc._state.prepend_free_semaphores(sem_nums)
        for poison_set in nc._tile_sem_poison_stack:
            poison_set.update(sem_nums)


def _custom_view(ap, dims, extra_offset=0):
    """Deep-copied AP with explicit [step, count] dims (overlap allowed)."""
    import copy
    v = copy.deepcopy(ap)
    v.ap = mybir.VecI64Pair([list(d) for d in dims])
    v.offset = v.offset + extra_offset
    return v


def _win_view(st, h, d0, nd):
    """AP over ST [260,134]: [wl=128 partitions, nd*134 contiguous floats]
    starting at column (128*h + d0) + wl.  The nd taps d0..d0+nd-1 of the
    column 5-window are one contiguous run per partition."""
    return _custom_view(st[:, :], [(134, 128), (1, nd * 134)],
                        extra_offset=(128 * h + d0) * 134)


def _build_nc():
    """One uniform SPMD program:
    in:  st [260,134] slab, ft [256,128] FBL, sm [128,256] = [S1 | S2]
    out: oacc [128,2] per-column partial sums (cols 0,127 garbage),
         oo4 [4,260] (outside at partitions 0,1,126,127 - the host derives
         the edge-column convs from it)."""
    nc = bass.Bass("TRN2", target_bir_lowering=False, debug=False,
                   num_devices=N_CORES)
    st = nc.declare_dram_parameter("st", [260, 134], BF16, isOutput=False)
    ft = nc.declare_dram_parameter("ft", [256, 128], F32, isOutput=False)
    sm = nc.declare_dram_parameter("sm", [128, 256], F32R, isOutput=False)
    oacc = nc.declare_dram_parameter("oacc", [128, 2], F32, isOutput=True)
    oo4 = nc.declare_dram_parameter("oo4", [4, 260], F32, isOutput=True)

    with _OneWaitTileContext(nc) as tc, ExitStack() as ctx:
        pool = ctx.enter_context(tc.tile_pool(name="sb", bufs=1))
        ppool = ctx.enter_context(tc.tile_pool(name="ps", bufs=1, space="PSUM"))

        # ---- input DMAs, 2-deep on SP and ACT (HWDGE); data usable ~1.7us
        # after the issue slice.  The slab is bf16: max-pooling bf16-rounded
        # inputs is exact w.r.t. the rounded values (5.6e-5 end-to-end error,
        # gate is 2e-2), and it halves both DMA bytes and DVE cycles (2x_1p
        # perf mode).  One 670-float window per half covers all 5 column
        # taps; the Pool/SWDGE queue stays empty (fewer sems to drain). ----
        L = pool.tile([128, 2 * 670], BF16, tag="L")
        Lv = L[:].rearrange("p (h r) -> p h r", h=2)
        F = pool.tile([128, 256], F32, tag="F")
        S = pool.tile([128, 256], F32R, tag="S")

        biasP = pool.tile([128, 1], F32, tag="biasP")
        nc.vector.memset(biasP[:], 5.0)                      # DVE tick 1

        nc.sync.dma_start(out=Lv[:, 0], in_=_win_view(st, 0, 0, 5))
        nc.scalar.dma_start(out=Lv[:, 1], in_=_win_view(st, 1, 0, 5))
        Fv = F[:].rearrange("p (h r) -> p h r", h=2)
        nc.sync.dma_start(
            out=Fv, in_=_custom_view(ft[:, :], [(128, 128), (128 * 128, 2),
                                                (1, 128)]))
        nc.scalar.dma_start(out=S[:], in_=sm[:, :])

        # pre-warm the sigmoid ACT table during the input loads
        warm = pool.tile([128, 1], F32, tag="warm")
        nc.scalar.activation(warm[:], biasP[:], AF.Sigmoid, bias=biasP[:])

        # ---- 5x5 max pool, both halves packed per op (DVE 2x bf16 mode).
        # The touch copy lets the first real op carry a single DMA wait. ----
        M = pool.tile([128, 260], BF16, tag="M")
        Mv = M[:].rearrange("p (h r) -> p h r", h=2)
        p01 = pool.tile([128, 2 * 134], BF16, tag="p01")
        p23 = pool.tile([128, 2 * 134], BF16, tag="p23")
        p03 = pool.tile([128, 2 * 134], BF16, tag="p03")
        cm = pool.tile([128, 2 * 134], BF16, tag="cm")
        r1 = pool.tile([128, 2 * 133], BF16, tag="r1")
        r2 = pool.tile([128, 2 * 131], BF16, tag="r2")
        p01v = p01[:].rearrange("p (h r) -> p h r", h=2)
        p23v = p23[:].rearrange("p (h r) -> p h r", h=2)
        p03v = p03[:].rearrange("p (h r) -> p h r", h=2)
        cmv = cm[:].rearrange("p (h r) -> p h r", h=2)
        r1v = r1[:].rearrange("p (h r) -> p h r", h=2)
        r2v = r2[:].rearrange("p (h r) -> p h r", h=2)

        t_a = pool.tile([1, 1], BF16, tag="t_a")
        nc.vector.tensor_copy(t_a[:], Lv[0:1, 0, 0:1])      # waits SP dma 1
        nc.vector.tensor_max(p01v, Lv[:, :, 0:134], Lv[:, :, 134:268])
        nc.vector.tensor_max(p23v, Lv[:, :, 268:402], Lv[:, :, 402:536])
        nc.vector.tensor_max(p03v, p01v, p23v)
        nc.vector.tensor_max(cmv, p03v, Lv[:, :, 536:670])
        # 5-tap row max (free dim): log tree
        nc.vector.tensor_max(r1v, cmv[:, :, 0:133], cmv[:, :, 1:134])
        nc.vector.tensor_max(r2v, r1v[:, :, 0:131], r1v[:, :, 2:133])
        nc.vector.tensor_max(Mv, r2v[:, :, 0:130], cmv[:, :, 4:134])

        # ---- outside = sigmoid(5 - 10*M), one ACT op; the natural-log
        # table swap for the later Ln overlaps the s2/PE-conv window ----
        o = pool.tile([128, 260], F32, tag="o")
        with nc.allow_low_precision(reason="f32r feed for PE conv matmul"):
            nc.scalar.activation(o[:].bitcast(F32R), M[:], AF.Sigmoid,
                                 bias=biasP[:], scale=-10.0)
        ov = o[:].rearrange("p (h r) -> p h r", h=2)
        oc = ov[:, :, 1:129]                       # [128, 2, 128] view

        # ---- row pair-sum s2 then conv = S1@s2 + S2@oc on PE/PSUM ----
        s2 = pool.tile([128, 256], F32, tag="s2")
        s2v = s2[:].rearrange("p (h r) -> p h r", h=2)
        with nc.allow_low_precision(reason="f32r feed for PE conv matmul"):
            nc.vector.tensor_add(s2v.bitcast(F32R),
                                 ov[:, :, 0:128], ov[:, :, 2:130])

        # G = outside*F on DVE while the PE convolves; walrus only runs
        # elementwise ops on DVE, so Pool/gpsimd is DMA-only.  The touch on
        # F's SWDGE data keeps G itself single-wait.
        t_f = pool.tile([1, 1], F32, tag="t_f")
        nc.vector.tensor_copy(t_f[:], F[0:1, 0:1])
        G = pool.tile([128, 256], F32, tag="G")
        Gv = G[:].rearrange("p (h r) -> p h r", h=2)
        nc.vector.tensor_mul(Gv, oc, Fv)

        psum = ppool.tile([128, 256], F32, tag="cv")
        psw = ppool.tile([1, 1], F32, tag="psw")
        # warm-up matmul: observes the S DMA on the PE stream (and spins up
        # the PE p-state) so the real matmuls carry only their DVE waits.
        nc.tensor.matmul(out=psw[:], lhsT=S[0:1, 0:1].bitcast(F32),
                         rhs=S[0:1, 1:2].bitcast(F32))
        # fp32r ISA restriction: moving/dst APs must be flat -> per-half mms.
        # Interleaved start/stop so only one PSUM accumulation group is open
        # at a time.
        for h in range(2):
            nc.tensor.matmul(out=psum[:, 128 * h:128 * h + 128],
                             lhsT=S[:, 128:256],
                             rhs=ov[:, h, 1:129].bitcast(F32R),
                             start=True, stop=False)
            nc.tensor.matmul(out=psum[:, 128 * h:128 * h + 128],
                             lhsT=S[:, 0:128],
                             rhs=s2[:, 128 * h:128 * h + 128].bitcast(F32R),
                             start=False, stop=True)

        # ---- dist*outside*F, accumulated per partition ----
        lnc = pool.tile([128, 256], F32, tag="lnc")
        nc.scalar.activation(lnc[:], psum[:], AF.Ln)
        v = pool.tile([128, 256], F32, tag="v")
        nc.vector.tensor_scalar_min(v[:], lnc[:], 0.0)
        junk = pool.tile([128, 256], F32, tag="junk")
        acc = pool.tile([128, 1], F32, tag="acc")
        # (-0.35 * min(ln,0)) * G == relu(-0.35*ln) * outside * F
        nc.vector.scalar_tensor_tensor(
            junk[:], v[:], -0.35, G[:], ALU.mult, ALU.mult, accum_out=acc[:])
        nc.sync.dma_start(out=oacc[:, 0:1], in_=acc[:])

        # ---- edge-row stores, off the critical path ----
        nc.sync.dma_start(out=oo4[0:2, :], in_=o[0:2, :])
        nc.sync.dma_start(out=oo4[2:4, :], in_=o[126:128, :])

    return nc


def _get_nc():
    global _NC_CACHE
    if _NC_CACHE is None:
        _NC_CACHE = _build_nc()
    return _NC_CACHE


def _shift_mats():
    sh = np.eye(128, k=1, dtype=np.float32) + np.eye(128, k=-1, dtype=np.float32)
    i128 = np.eye(128, dtype=np.float32)
    s1 = np.float32(KB) * sh + np.float32(KA) * i128
    s2 = np.float32(KA) * sh + i128
    return np.ascontiguousarray(np.concatenate([s1, s2], axis=1))  # [128,256]


def _prep_in_maps(pred, target):
    import ml_dtypes
    pred = np.asarray(pred, np.float32)
    target = np.asarray(target, np.float32)
    sm = _shift_mats()
    in_maps = []
    for c in range(N_CORES):
        b, h = c // 2, c % 2
        r0 = 128 * h
        lm = target[b, 0]                                    # [256,256]
        S = np.full((134, 260), NEG, np.float32)
        lo, hi = max(0, r0 - 3), min(H, r0 + 131)
        S[lo - (r0 - 3): hi - (r0 - 3), 2:258] = lm[lo:hi]
        if h == 0:
            S[0, 2:258] = lm[2]      # fictitious row -3 := row 2 (replicate)
        else:
            S[133, 2:258] = lm[253]  # fictitious row 258 := row 253
        ST = np.ascontiguousarray(S.T).astype(ml_dtypes.bfloat16)  # [260,134]
        FT = np.ascontiguousarray(pred[b, 1, r0:r0 + 128, :].T)  # [256,128]
        in_maps.append({"st": ST, "ft": FT, "sm": sm})
    return in_maps


def _combine(core_outs, in_maps):
    """Interior column sums from the device + host-recomputed edge columns
    (w = 0, 127, 128, 255 per core, where the partition shift wraps)."""
    ka, kb = np.float32(KA), np.float32(KB)
    total = 0.0
    for c in range(N_CORES):
        r = core_outs[c]
        acc = np.asarray(r["oacc"], np.float32).sum(axis=1)
        O4 = np.asarray(r["oo4"], np.float32)    # partitions [0,1,126,127]
        FT = in_maps[c]["ft"]                    # [256,128]
        total += float(np.sum(acc[1:127].astype(np.float64)))
        # derive P/Q rows from o rows (per-partition free-dim 3-tap convs)
        PQ = {}
        for row, part in ((0, 0), (1, 1), (2, 126), (3, 127)):
            Prow = np.empty(256, np.float32)
            Qrow = np.empty(256, np.float32)
            Orow = np.empty((2, 128), np.float32)
            for h in range(2):
                oh = O4[row, 130 * h: 130 * h + 130]
                s2 = oh[0:128] + oh[2:130]
                ocr = oh[1:129]
                Prow[128 * h:128 * h + 128] = kb * s2 + ka * ocr
                Qrow[128 * h:128 * h + 128] = ka * s2 + ocr
                Orow[h] = ocr
            PQ[part] = (Prow, Qrow, Orow)
        for h in range(2):
            col = 128 * h
            Ph = lambda part, hh: PQ[part][0][128 * hh: 128 * hh + 128]
            # wl = 0:  conv = P[w-1] + Q[w] + P[w+1]
            left = Ph(0, 0) if h == 0 else Ph(127, 0)    # replicate / stitch
            conv0 = left + PQ[0][1][col:col + 128] + Ph(1, h)
            # wl = 127
            right = Ph(0, 1) if h == 0 else Ph(127, 1)
            conv127 = Ph(126, h) + PQ[127][1][col:col + 128] + right
            for wl, conv in ((0, conv0), (127, conv127)):
                cdtr = np.maximum(np.float32(-0.35) * np.log(conv), 0.0)
                pen = np.minimum(cdtr, 10.0)
                ocr = PQ[wl][2][h]
                Fr = FT[128 * h + wl]
                total += float(np.sum((pen * ocr * Fr).astype(np.float64)))
    return np.float32(total / (10.0 * B * H * W))


def _run(pred, target, trace=False, **kw):
    nc = _get_nc()
    in_maps = _prep_in_maps(pred, target)
    res = run_bass_kernel_spmd(nc, in_maps, list(range(N_CORES)),
                               trace=trace, **kw)
    value = _combine(res.results, in_maps)
    return value, res


def kernel(pred, target):
    value, _ = _run(pred, target)
    return value
